# revision 3
# baseline (speedup 1.0000x reference)
"""Cosine VQ (codebook) kernel for Trainium2, 8 NeuronCores, data-parallel over N.

Strategy per core (N_shard = 4096 latent rows, full codebook replicated):
  - normalize codebook rows on device (square -> windowed-reduce -> sqrt -> recip -> scale)
  - transpose latents and normalized codebook to d-major via PE transposes
  - sim tiles [128 latents x 8192 codes] via fp32 PE matmuls (exact fp32 class)
  - argmax per row via hardware InstMax (top-8) + InstMaxIndex
  - indices DMA'd out; host gathers codebook rows, computes straight-through
    output and the scalar loss from per-core partials (the "all-reduce").
"""
import os
import sys
import types

import numpy as np

_TRN_REPO = "/opt/trn_rl_repo"
if _TRN_REPO not in sys.path:
    sys.path.insert(0, _TRN_REPO)

import concourse.bass as bass
import concourse.mybir as mybir
from concourse import tile
from concourse.tile import ScopedClock

N_CORES = 8
N_FULL = 1024 * 32          # 32768 latent rows
N_SHARD = N_FULL // N_CORES  # 4096
D = 64
N_E = 8192
N_TILES = N_SHARD // 128     # 32 latent tiles per core
EPS = 1e-12
BETA = 0.25


def _install_profhook():
    """Re-register the NTFF profile hook trn_boot skips (antenv.axon_hooks
    is absent in this image) and stub the S3 artifact upload."""
    import antenv
    if "antenv.axon_hooks" not in sys.modules:
        mod = types.ModuleType("antenv.axon_hooks")
        _hook = [None]
        mod.set_axon_ntff_profile_hook = lambda h: _hook.__setitem__(0, h)
        mod.get_axon_ntff_profile_hook = lambda: _hook[0]
        sys.modules["antenv.axon_hooks"] = mod
        antenv.axon_hooks = mod
        try:
            from trn_agent_boot.trn_boot import _ntff_profile_via_ctypes
            mod.set_axon_ntff_profile_hook(
                _ntff_profile_via_ctypes("/opt/axon/libaxon_pjrt.so"))
        except Exception:
            pass
    import concourse.bass_utils as bu
    bu.upload_artifacts = lambda tmpdir: tmpdir


def _install_tilepatch():
    """This walrus build allows only ONE sync wait per TPB instruction; Tile's
    tail drain carries one wait per outstanding proc. Split them onto
    single-wait SP nops."""
    def _drain_and_barrier(self, tick_clock, wait_clock):
        nc = self.nc
        drain_inst = nc.sync.drain()
        wait_clock.add_sem_waits(
            drain_inst.ins, ScopedClock({None: tick_clock.global_clock}))
        si = drain_inst.ins.sync_info
        waits = list(si.on_wait or [])
        if len(waits) > 1:
            bb = nc.cur_bb.bb
            idx = bb.instructions.index(drain_inst.ins)
            nops = []
            for w in waits[:-1]:
                n = nc.sync.nop(nofuse=True)
                n.ins.sync_info = mybir.SyncInfo(on_wait=[w], on_update=[])
                nops.append(n.ins)
            si.on_wait = [waits[-1]]
            for n in nops:
                bb.instructions.remove(n)
            bb.instructions[idx:idx] = nops
        nc.all_engine_barrier()
        assert self.sems is not None
        popped = nc._tile_sem_poison_stack.pop()
        assert popped is self._sem_poison
        nc.clear_and_free_semaphores(list(self.sems.allocated().values()))
        nc.all_engine_barrier()

    tile.TileContext._drain_and_barrier = _drain_and_barrier


def _split_waits(nc, max_waits=1):
    """Hoist extra sync waits onto same-engine single-wait NOPs (same-engine
    FIFO makes this equivalent)."""
    n_split = 0
    for f in nc.m.functions:
        for bb in f.blocks:
            new_list = []
            for ins in bb.instructions:
                si = ins.sync_info
                waits = list(si.on_wait) if si and si.on_wait else []
                if len(waits) > max_waits:
                    eng = nc.engines[ins.engine]
                    extra, keep = waits[:-max_waits], waits[-max_waits:]
                    for w in extra:
                        nop = eng.nop(nofuse=True)
                        nopi = nop.ins
                        for bb2 in f.blocks:
                            if nopi in bb2.instructions:
                                bb2.instructions.remove(nopi)
                                break
                        nopi.sync_info = mybir.SyncInfo(on_wait=[w], on_update=[])
                        new_list.append(nopi)
                        n_split += 1
                    si.on_wait = keep
                new_list.append(ins)
            bb.instructions[:] = new_list
    return n_split


def _build_module():
    A = mybir.AluOpType
    f32 = mybir.dt.float32
    nc = bass.Bass()
    x_d = nc.dram_tensor("x", [N_SHARD, D], f32, kind="ExternalInput")
    emb_d = nc.dram_tensor("emb", [N_E, D], f32, kind="ExternalInput")
    id_d = nc.dram_tensor("ident", [128, 128], f32, kind="ExternalInput")
    idx_d = nc.dram_tensor("idx", [N_SHARD, 1], mybir.dt.int32,
                           kind="ExternalOutput")

    with tile.TileContext(nc) as tc:
        with tc.tile_pool(name="sb", bufs=1) as pool:
            ident = pool.tile([128, 128], f32)
            nc.sync.dma_start(ident[:], id_d[:])

            # ---- load emb as [128, 64*64] (tile t at cols t*64:(t+1)*64) ----
            emb_sb = pool.tile([128, 64 * D], f32)
            nc.sync.dma_start(
                emb_sb[:].rearrange("p (t d) -> p t d", t=64),
                emb_d[:].rearrange("(t p) d -> p t d", p=128))
            # ---- load x as [128, 32*64] ----
            x_sb = pool.tile([128, N_TILES * D], f32)
            nc.sync.dma_start(
                x_sb[:].rearrange("p (t d) -> p t d", t=N_TILES),
                x_d[:].rearrange("(t p) d -> p t d", p=128))

            # ---- codebook norms ----
            sq = pool.tile([128, 64 * D], f32)
            nc.scalar.activation(sq[:], emb_sb[:],
                                 mybir.ActivationFunctionType.Square)
            n2 = pool.tile([128, 64], f32)
            nc.vector.tensor_reduce(
                n2[:], sq[:].rearrange("p (t d) -> p t d", t=64),
                op=A.add, axis=mybir.AxisListType.X)
            nrm = pool.tile([128, 64], f32)
            nc.scalar.activation(nrm[:], n2[:],
                                 mybir.ActivationFunctionType.Sqrt)
            nc.vector.tensor_scalar_max(nrm[:], nrm[:], EPS)
            rn = pool.tile([128, 64], f32)
            nc.vector.reciprocal(rn[:], nrm[:])

            # ---- normalize + transpose codebook -> cbT [64, 8192] ----
            cbT = pool.tile([64, N_E], f32)
            latT = pool.tile([64, N_TILES * 128], f32)
            with tc.tile_pool(name="pst", bufs=4, space="PSUM") as pstpool:
                for t in range(64):
                    cn = pool.tile([128, D], f32, name=f"cn{t%4}", tag="cn", bufs=4)
                    nc.vector.tensor_scalar_mul(cn[:], emb_sb[:, t * D:(t + 1) * D],
                                                rn[:, t:t + 1])
                    pt = pstpool.tile([64, 128], f32, name=f"pt{t%4}", tag="pt")
                    nc.tensor.transpose(pt[:], cn[:], ident[:])
                    nc.scalar.copy(cbT[:, t * 128:(t + 1) * 128], pt[:])

                # ---- transpose latents -> latT [64, 32*128] ----
                for i in range(N_TILES):
                    pt2 = pstpool.tile([64, 128], f32, name=f"pt2{i%4}", tag="pt")
                    nc.tensor.transpose(pt2[:], x_sb[:, i * D:(i + 1) * D], ident[:])
                    nc.scalar.copy(latT[:, i * 128:(i + 1) * 128], pt2[:])

            # ---- main loop: sims + argmax ----
            sims = pool.tile([128, N_E], f32, bufs=2)
            with tc.tile_pool(name="ps", bufs=2, space="PSUM") as pspool:
              for i in range(N_TILES):
                lhsT = latT[:, i * 128:(i + 1) * 128]
                for r in range(4):
                    ps = pspool.tile([128, 2048], f32, name=f"ps{r%2}",
                                     tag="psum_mm")
                    for j in range(4):
                        c = r * 4 + j
                        nc.tensor.matmul(ps[:, j * 512:(j + 1) * 512], lhsT,
                                         cbT[:, c * 512:(c + 1) * 512],
                                         start=True, stop=True)
                    if r % 2 == 0:
                        nc.scalar.copy(sims[:, r * 2048:(r + 1) * 2048], ps[:])
                    else:
                        nc.vector.tensor_copy(sims[:, r * 2048:(r + 1) * 2048],
                                              ps[:])
                mx8 = pool.tile([128, 8], f32, name=f"mx8_{i%4}", tag="mx8",
                                bufs=4)
                ix8 = pool.tile([128, 8], mybir.dt.uint32, name=f"ix8_{i%4}",
                                tag="ix8", bufs=4)
                nc.vector.max(mx8[:], sims[:])
                nc.vector.max_index(ix8[:], mx8[:], sims[:])
                nc.sync.dma_start(idx_d[i * 128:(i + 1) * 128, :],
                                  ix8[:, 0:1].bitcast(mybir.dt.int32))

    return nc


_CACHE = {}


def _get_kernel():
    if "nc" not in _CACHE:
        _install_profhook()
        _install_tilepatch()
        nc = _build_module()
        _split_waits(nc)
        _CACHE["nc"] = nc
    return _CACHE["nc"]


last_results = None


def kernel(x: np.ndarray, emb_weight: np.ndarray):
    global last_results
    from concourse.bass_utils import run_bass_kernel_spmd

    nc = _get_kernel()
    x = np.asarray(x, dtype=np.float32)
    emb = np.asarray(emb_weight, dtype=np.float32)
    flat = x.reshape(-1, D)
    ident = np.eye(128, dtype=np.float32)
    in_maps = [
        {"x": np.ascontiguousarray(flat[c * N_SHARD:(c + 1) * N_SHARD]),
         "emb": emb, "ident": ident}
        for c in range(N_CORES)
    ]
    trace = bool(os.environ.get("VQ_TRACE"))
    res = run_bass_kernel_spmd(nc, in_maps, core_ids=list(range(N_CORES)),
                               trace=trace)
    last_results = res
    idx = np.concatenate([res.results[c]["idx"][:, 0] for c in range(N_CORES)])
    idx = idx.astype(np.int32)

    codebook_vec = emb[idx].reshape(x.shape)
    x_q_st = x + (codebook_vec - x)
    diff = codebook_vec - x
    mse = np.float32(np.mean((diff * diff).astype(np.float32), dtype=np.float32))
    loss = np.float32(mse + np.float32(BETA) * mse)
    indices_out = idx.reshape(x.shape[:-1])
    return (x_q_st, loss, indices_out, codebook_vec)


# revision 4
# speedup vs baseline: 1.1900x; 1.1900x over previous
"""Cosine VQ (codebook) kernel for Trainium2, 8 NeuronCores, data-parallel over N.

Strategy per core (N_shard = 4096 latent rows, full codebook replicated):
  - normalize codebook rows on device (square -> windowed-reduce -> sqrt -> recip -> scale)
  - transpose latents and normalized codebook to d-major via PE transposes
  - sim tiles [128 latents x 8192 codes] via fp32 PE matmuls (exact fp32 class)
  - argmax per row via hardware InstMax (top-8) + InstMaxIndex
  - indices DMA'd out; host gathers codebook rows, computes straight-through
    output and the scalar loss from per-core partials (the "all-reduce").
"""
import os
import sys
import types

import numpy as np

_TRN_REPO = "/opt/trn_rl_repo"
if _TRN_REPO not in sys.path:
    sys.path.insert(0, _TRN_REPO)

import concourse.bass as bass
import concourse.mybir as mybir
from concourse import tile
from concourse.tile import ScopedClock

N_CORES = 8
N_FULL = 1024 * 32          # 32768 latent rows
N_SHARD = N_FULL // N_CORES  # 4096
D = 64
N_E = 8192
N_TILES = N_SHARD // 128     # 32 latent tiles per core
EPS = 1e-12
BETA = 0.25


def _install_profhook():
    """Re-register the NTFF profile hook trn_boot skips (antenv.axon_hooks
    is absent in this image) and stub the S3 artifact upload."""
    import antenv
    if "antenv.axon_hooks" not in sys.modules:
        mod = types.ModuleType("antenv.axon_hooks")
        _hook = [None]
        mod.set_axon_ntff_profile_hook = lambda h: _hook.__setitem__(0, h)
        mod.get_axon_ntff_profile_hook = lambda: _hook[0]
        sys.modules["antenv.axon_hooks"] = mod
        antenv.axon_hooks = mod
        try:
            from trn_agent_boot.trn_boot import _ntff_profile_via_ctypes
            mod.set_axon_ntff_profile_hook(
                _ntff_profile_via_ctypes("/opt/axon/libaxon_pjrt.so"))
        except Exception:
            pass
    import concourse.bass_utils as bu
    bu.upload_artifacts = lambda tmpdir: tmpdir


def _install_tilepatch():
    """This walrus build allows only ONE sync wait per TPB instruction; Tile's
    tail drain carries one wait per outstanding proc. Split them onto
    single-wait SP nops."""
    def _drain_and_barrier(self, tick_clock, wait_clock):
        nc = self.nc
        drain_inst = nc.sync.drain()
        wait_clock.add_sem_waits(
            drain_inst.ins, ScopedClock({None: tick_clock.global_clock}))
        si = drain_inst.ins.sync_info
        waits = list(si.on_wait or [])
        if len(waits) > 1:
            bb = nc.cur_bb.bb
            idx = bb.instructions.index(drain_inst.ins)
            nops = []
            for w in waits[:-1]:
                n = nc.sync.nop(nofuse=True)
                n.ins.sync_info = mybir.SyncInfo(on_wait=[w], on_update=[])
                nops.append(n.ins)
            si.on_wait = [waits[-1]]
            for n in nops:
                bb.instructions.remove(n)
            bb.instructions[idx:idx] = nops
        nc.all_engine_barrier()
        assert self.sems is not None
        popped = nc._tile_sem_poison_stack.pop()
        assert popped is self._sem_poison
        nc.clear_and_free_semaphores(list(self.sems.allocated().values()))
        nc.all_engine_barrier()

    tile.TileContext._drain_and_barrier = _drain_and_barrier


def _split_waits(nc, max_waits=1):
    """Hoist extra sync waits onto same-engine single-wait NOPs (same-engine
    FIFO makes this equivalent)."""
    n_split = 0
    for f in nc.m.functions:
        for bb in f.blocks:
            new_list = []
            for ins in bb.instructions:
                si = ins.sync_info
                waits = list(si.on_wait) if si and si.on_wait else []
                if len(waits) > max_waits:
                    eng = nc.engines[ins.engine]
                    extra, keep = waits[:-max_waits], waits[-max_waits:]
                    for w in extra:
                        nop = eng.nop(nofuse=True)
                        nopi = nop.ins
                        for bb2 in f.blocks:
                            if nopi in bb2.instructions:
                                bb2.instructions.remove(nopi)
                                break
                        nopi.sync_info = mybir.SyncInfo(on_wait=[w], on_update=[])
                        new_list.append(nopi)
                        n_split += 1
                    si.on_wait = keep
                new_list.append(ins)
            bb.instructions[:] = new_list
    return n_split


def _build_module():
    A = mybir.AluOpType
    f32 = mybir.dt.float32
    nc = bass.Bass()
    x_d = nc.dram_tensor("x", [N_SHARD, D], f32, kind="ExternalInput")
    emb_d = nc.dram_tensor("emb", [N_E, D], f32, kind="ExternalInput")
    id_d = nc.dram_tensor("ident", [128, 128], f32, kind="ExternalInput")
    idx_d = nc.dram_tensor("idx", [N_SHARD, 1], mybir.dt.int32,
                           kind="ExternalOutput")

    with tile.TileContext(nc) as tc:
        with tc.tile_pool(name="sb", bufs=1) as pool:
            ident = pool.tile([128, 128], f32)
            nc.sync.dma_start(ident[:], id_d[:])

            # ---- load emb as [128, 64*64] (tile t at cols t*64:(t+1)*64) ----
            emb_sb = pool.tile([128, 64 * D], f32)
            nc.sync.dma_start(
                emb_sb[:].rearrange("p (t d) -> p t d", t=64),
                emb_d[:].rearrange("(t p) d -> p t d", p=128))
            # ---- load x as [128, 32*64] ----
            x_sb = pool.tile([128, N_TILES * D], f32)
            nc.sync.dma_start(
                x_sb[:].rearrange("p (t d) -> p t d", t=N_TILES),
                x_d[:].rearrange("(t p) d -> p t d", p=128))

            # ---- codebook norms ----
            sq = pool.tile([128, 64 * D], f32)
            nc.scalar.activation(sq[:], emb_sb[:],
                                 mybir.ActivationFunctionType.Square)
            n2 = pool.tile([128, 64], f32)
            nc.vector.tensor_reduce(
                n2[:], sq[:].rearrange("p (t d) -> p t d", t=64),
                op=A.add, axis=mybir.AxisListType.X)
            nrm = pool.tile([128, 64], f32)
            nc.scalar.activation(nrm[:], n2[:],
                                 mybir.ActivationFunctionType.Sqrt)
            nc.vector.tensor_scalar_max(nrm[:], nrm[:], EPS)
            rn = pool.tile([128, 64], f32)
            nc.vector.reciprocal(rn[:], nrm[:])

            # ---- normalize + transpose codebook -> cbT [64, 8192] ----
            cbT = pool.tile([64, N_E], f32)
            latT = pool.tile([64, N_TILES * 128], f32)
            with tc.tile_pool(name="pst", bufs=4, space="PSUM") as pstpool:
                for t in range(64):
                    cn = pool.tile([128, D], f32, name=f"cn{t%4}", tag="cn", bufs=4)
                    nc.vector.tensor_scalar_mul(cn[:], emb_sb[:, t * D:(t + 1) * D],
                                                rn[:, t:t + 1])
                    pt = pstpool.tile([64, 128], f32, name=f"pt{t%4}", tag="pt")
                    nc.tensor.transpose(pt[:], cn[:], ident[:])
                    nc.scalar.copy(cbT[:, t * 128:(t + 1) * 128], pt[:])

                # ---- transpose latents -> latT [64, 32*128] ----
                for i in range(N_TILES):
                    pt2 = pstpool.tile([64, 128], f32, name=f"pt2{i%4}", tag="pt")
                    nc.tensor.transpose(pt2[:], x_sb[:, i * D:(i + 1) * D], ident[:])
                    nc.scalar.copy(latT[:, i * 128:(i + 1) * 128], pt2[:])

            # ---- main loop: sims + argmax ----
            with tc.tile_pool(name="ps", bufs=2, space="PSUM") as pspool:
              for i in range(N_TILES):
                sims = pool.tile([128, N_E], f32, name=f"sims{i % 2}",
                                 tag="sims", bufs=2)
                lhsT = latT[:, i * 128:(i + 1) * 128]
                for r in range(4):
                    ps = pspool.tile([128, 2048], f32, name=f"ps{r%2}",
                                     tag="psum_mm")
                    for j in range(4):
                        c = r * 4 + j
                        nc.tensor.matmul(ps[:, j * 512:(j + 1) * 512], lhsT,
                                         cbT[:, c * 512:(c + 1) * 512],
                                         start=True, stop=True)
                    if r < 3:
                        nc.scalar.copy(sims[:, r * 2048:(r + 1) * 2048], ps[:])
                    else:
                        nc.vector.tensor_copy(sims[:, r * 2048:(r + 1) * 2048],
                                              ps[:])
                mx8 = pool.tile([128, 8], f32, name=f"mx8_{i%4}", tag="mx8",
                                bufs=4)
                ix8 = pool.tile([128, 8], mybir.dt.uint32, name=f"ix8_{i%4}",
                                tag="ix8", bufs=4)
                nc.vector.max(mx8[:], sims[:])
                nc.vector.max_index(ix8[:], mx8[:], sims[:])
                nc.sync.dma_start(idx_d[i * 128:(i + 1) * 128, :],
                                  ix8[:, 0:1].bitcast(mybir.dt.int32))

    return nc


_CACHE = {}


def _get_kernel():
    if "nc" not in _CACHE:
        _install_profhook()
        _install_tilepatch()
        nc = _build_module()
        _split_waits(nc)
        _CACHE["nc"] = nc
    return _CACHE["nc"]


last_results = None


def kernel(x: np.ndarray, emb_weight: np.ndarray):
    global last_results
    from concourse.bass_utils import run_bass_kernel_spmd

    nc = _get_kernel()
    x = np.asarray(x, dtype=np.float32)
    emb = np.asarray(emb_weight, dtype=np.float32)
    flat = x.reshape(-1, D)
    ident = np.eye(128, dtype=np.float32)
    in_maps = [
        {"x": np.ascontiguousarray(flat[c * N_SHARD:(c + 1) * N_SHARD]),
         "emb": emb, "ident": ident}
        for c in range(N_CORES)
    ]
    trace = bool(os.environ.get("VQ_TRACE"))
    res = run_bass_kernel_spmd(nc, in_maps, core_ids=list(range(N_CORES)),
                               trace=trace)
    last_results = res
    idx = np.concatenate([res.results[c]["idx"][:, 0] for c in range(N_CORES)])
    idx = idx.astype(np.int32)

    codebook_vec = emb[idx].reshape(x.shape)
    x_q_st = x + (codebook_vec - x)
    diff = codebook_vec - x
    mse = np.float32(np.mean((diff * diff).astype(np.float32), dtype=np.float32))
    loss = np.float32(mse + np.float32(BETA) * mse)
    indices_out = idx.reshape(x.shape[:-1])
    return (x_q_st, loss, indices_out, codebook_vec)


# revision 5
# speedup vs baseline: 1.2828x; 1.0780x over previous
"""Cosine VQ (codebook) kernel for Trainium2, 8 NeuronCores, data-parallel over N.

Strategy per core (N_shard = 4096 latent rows, full codebook replicated):
  - normalize codebook rows on device (square -> windowed-reduce -> sqrt -> recip -> scale)
  - transpose latents and normalized codebook to d-major via PE transposes
  - sim tiles [128 latents x 8192 codes] via fp32 PE matmuls (exact fp32 class)
  - argmax per row via hardware InstMax (top-8) + InstMaxIndex
  - indices DMA'd out; host gathers codebook rows, computes straight-through
    output and the scalar loss from per-core partials (the "all-reduce").
"""
import os
import sys
import types

import numpy as np

_TRN_REPO = "/opt/trn_rl_repo"
if _TRN_REPO not in sys.path:
    sys.path.insert(0, _TRN_REPO)

import concourse.bass as bass
import concourse.mybir as mybir
from concourse import tile
from concourse.tile import ScopedClock

N_CORES = 8
N_FULL = 1024 * 32          # 32768 latent rows
N_SHARD = N_FULL // N_CORES  # 4096
D = 64
N_E = 8192
N_TILES = N_SHARD // 128     # 32 latent tiles per core
EPS = 1e-12
BETA = 0.25


def _install_profhook():
    """Re-register the NTFF profile hook trn_boot skips (antenv.axon_hooks
    is absent in this image) and stub the S3 artifact upload."""
    import antenv
    if "antenv.axon_hooks" not in sys.modules:
        mod = types.ModuleType("antenv.axon_hooks")
        _hook = [None]
        mod.set_axon_ntff_profile_hook = lambda h: _hook.__setitem__(0, h)
        mod.get_axon_ntff_profile_hook = lambda: _hook[0]
        sys.modules["antenv.axon_hooks"] = mod
        antenv.axon_hooks = mod
        try:
            from trn_agent_boot.trn_boot import _ntff_profile_via_ctypes
            mod.set_axon_ntff_profile_hook(
                _ntff_profile_via_ctypes("/opt/axon/libaxon_pjrt.so"))
        except Exception:
            pass
    import concourse.bass_utils as bu
    bu.upload_artifacts = lambda tmpdir: tmpdir


def _install_tilepatch():
    """This walrus build allows only ONE sync wait per TPB instruction; Tile's
    tail drain carries one wait per outstanding proc. Split them onto
    single-wait SP nops."""
    def _drain_and_barrier(self, tick_clock, wait_clock):
        nc = self.nc
        drain_inst = nc.sync.drain()
        wait_clock.add_sem_waits(
            drain_inst.ins, ScopedClock({None: tick_clock.global_clock}))
        si = drain_inst.ins.sync_info
        waits = list(si.on_wait or [])
        if len(waits) > 1:
            bb = nc.cur_bb.bb
            idx = bb.instructions.index(drain_inst.ins)
            nops = []
            for w in waits[:-1]:
                n = nc.sync.nop(nofuse=True)
                n.ins.sync_info = mybir.SyncInfo(on_wait=[w], on_update=[])
                nops.append(n.ins)
            si.on_wait = [waits[-1]]
            for n in nops:
                bb.instructions.remove(n)
            bb.instructions[idx:idx] = nops
        nc.all_engine_barrier()
        assert self.sems is not None
        popped = nc._tile_sem_poison_stack.pop()
        assert popped is self._sem_poison
        nc.clear_and_free_semaphores(list(self.sems.allocated().values()))
        nc.all_engine_barrier()

    tile.TileContext._drain_and_barrier = _drain_and_barrier


def _split_waits(nc, max_waits=1):
    """Hoist extra sync waits onto same-engine single-wait NOPs (same-engine
    FIFO makes this equivalent)."""
    n_split = 0
    for f in nc.m.functions:
        for bb in f.blocks:
            new_list = []
            for ins in bb.instructions:
                si = ins.sync_info
                waits = list(si.on_wait) if si and si.on_wait else []
                if len(waits) > max_waits:
                    eng = nc.engines[ins.engine]
                    extra, keep = waits[:-max_waits], waits[-max_waits:]
                    for w in extra:
                        nop = eng.nop(nofuse=True)
                        nopi = nop.ins
                        for bb2 in f.blocks:
                            if nopi in bb2.instructions:
                                bb2.instructions.remove(nopi)
                                break
                        nopi.sync_info = mybir.SyncInfo(on_wait=[w], on_update=[])
                        new_list.append(nopi)
                        n_split += 1
                    si.on_wait = keep
                new_list.append(ins)
            bb.instructions[:] = new_list
    return n_split


def _build_module():
    A = mybir.AluOpType
    f32 = mybir.dt.float32
    nc = bass.Bass()
    x_d = nc.dram_tensor("x", [N_SHARD, D], f32, kind="ExternalInput")
    emb_d = nc.dram_tensor("emb", [N_E, D], f32, kind="ExternalInput")
    id_d = nc.dram_tensor("ident", [128, 128], f32, kind="ExternalInput")
    idx_d = nc.dram_tensor("idx", [N_SHARD, 1], mybir.dt.int32,
                           kind="ExternalOutput")

    with tile.TileContext(nc) as tc:
        with tc.tile_pool(name="sb", bufs=1) as pool:
            ident = pool.tile([128, 128], f32)
            nc.sync.dma_start(ident[:], id_d[:])

            # ---- load emb as [128, 64*64] (tile t at cols t*64:(t+1)*64) ----
            emb_sb = pool.tile([128, 64 * D], f32)
            nc.sync.dma_start(
                emb_sb[:].rearrange("p (t d) -> p t d", t=64),
                emb_d[:].rearrange("(t p) d -> p t d", p=128))
            # ---- load x as [128, 32*64] ----
            x_sb = pool.tile([128, N_TILES * D], f32)
            nc.sync.dma_start(
                x_sb[:].rearrange("p (t d) -> p t d", t=N_TILES),
                x_d[:].rearrange("(t p) d -> p t d", p=128))

            # ---- codebook norms ----
            sq = pool.tile([128, 64 * D], f32)
            nc.scalar.activation(sq[:], emb_sb[:],
                                 mybir.ActivationFunctionType.Square)
            n2 = pool.tile([128, 64], f32)
            nc.vector.tensor_reduce(
                n2[:], sq[:].rearrange("p (t d) -> p t d", t=64),
                op=A.add, axis=mybir.AxisListType.X)
            nrm = pool.tile([128, 64], f32)
            nc.scalar.activation(nrm[:], n2[:],
                                 mybir.ActivationFunctionType.Sqrt)
            nc.vector.tensor_scalar_max(nrm[:], nrm[:], EPS)
            rn = pool.tile([128, 64], f32)
            nc.vector.reciprocal(rn[:], nrm[:])

            # ---- normalize + transpose codebook -> cbT [64, 8192] ----
            cbT = pool.tile([64, N_E], f32)
            latT = pool.tile([64, N_TILES * 128], f32)
            with tc.tile_pool(name="pst", bufs=4, space="PSUM") as pstpool:
                for t in range(64):
                    cn = pool.tile([128, D], f32, name=f"cn{t%4}", tag="cn", bufs=4)
                    nc.vector.tensor_scalar_mul(cn[:], emb_sb[:, t * D:(t + 1) * D],
                                                rn[:, t:t + 1])
                    pt = pstpool.tile([64, 128], f32, name=f"pt{t%4}", tag="pt")
                    nc.tensor.transpose(pt[:], cn[:], ident[:])
                    nc.scalar.copy(cbT[:, t * 128:(t + 1) * 128], pt[:])

                # ---- transpose latents -> latT [64, 32*128] ----
                for i in range(N_TILES):
                    pt2 = pstpool.tile([64, 128], f32, name=f"pt2{i%4}", tag="pt")
                    nc.tensor.transpose(pt2[:], x_sb[:, i * D:(i + 1) * D], ident[:])
                    nc.scalar.copy(latT[:, i * 128:(i + 1) * 128], pt2[:])

            # ---- main loop: sims + argmax ----
            with tc.tile_pool(name="ps", bufs=4, space="PSUM") as pspool:
              for i in range(N_TILES):
                sims = pool.tile([128, N_E], f32, name=f"sims{i % 2}",
                                 tag="sims", bufs=2)
                lhsT = latT[:, i * 128:(i + 1) * 128]
                for r in range(8):
                    ps = pspool.tile([128, 1024], f32, name=f"ps{r%4}",
                                     tag="psum_mm", bufs=4)
                    for j in range(2):
                        c = r * 2 + j
                        nc.tensor.matmul(ps[:, j * 512:(j + 1) * 512], lhsT,
                                         cbT[:, c * 512:(c + 1) * 512],
                                         start=True, stop=True)
                    if r % 4 != 3:
                        nc.scalar.copy(sims[:, r * 1024:(r + 1) * 1024], ps[:])
                    else:
                        nc.vector.tensor_copy(sims[:, r * 1024:(r + 1) * 1024],
                                              ps[:])
                mx8 = pool.tile([128, 8], f32, name=f"mx8_{i%4}", tag="mx8",
                                bufs=4)
                ix8 = pool.tile([128, 8], mybir.dt.uint32, name=f"ix8_{i%4}",
                                tag="ix8", bufs=4)
                nc.vector.max(mx8[:], sims[:])
                nc.vector.max_index(ix8[:], mx8[:], sims[:])
                nc.sync.dma_start(idx_d[i * 128:(i + 1) * 128, :],
                                  ix8[:, 0:1].bitcast(mybir.dt.int32))

    return nc


_CACHE = {}


def _get_kernel():
    if "nc" not in _CACHE:
        _install_profhook()
        _install_tilepatch()
        nc = _build_module()
        _split_waits(nc)
        _CACHE["nc"] = nc
    return _CACHE["nc"]


last_results = None


def kernel(x: np.ndarray, emb_weight: np.ndarray):
    global last_results
    from concourse.bass_utils import run_bass_kernel_spmd

    nc = _get_kernel()
    x = np.asarray(x, dtype=np.float32)
    emb = np.asarray(emb_weight, dtype=np.float32)
    flat = x.reshape(-1, D)
    ident = np.eye(128, dtype=np.float32)
    in_maps = [
        {"x": np.ascontiguousarray(flat[c * N_SHARD:(c + 1) * N_SHARD]),
         "emb": emb, "ident": ident}
        for c in range(N_CORES)
    ]
    trace = bool(os.environ.get("VQ_TRACE"))
    res = run_bass_kernel_spmd(nc, in_maps, core_ids=list(range(N_CORES)),
                               trace=trace)
    last_results = res
    idx = np.concatenate([res.results[c]["idx"][:, 0] for c in range(N_CORES)])
    idx = idx.astype(np.int32)

    codebook_vec = emb[idx].reshape(x.shape)
    x_q_st = x + (codebook_vec - x)
    diff = codebook_vec - x
    mse = np.float32(np.mean((diff * diff).astype(np.float32), dtype=np.float32))
    loss = np.float32(mse + np.float32(BETA) * mse)
    indices_out = idx.reshape(x.shape[:-1])
    return (x_q_st, loss, indices_out, codebook_vec)


# revision 6
# speedup vs baseline: 1.6935x; 1.3202x over previous
"""Cosine VQ (codebook) kernel for Trainium2, 8 NeuronCores, data-parallel over N.

Strategy per core (N_shard = 4096 latent rows, full codebook replicated):
  - normalize codebook rows on device (square -> windowed-reduce -> sqrt -> recip -> scale)
  - transpose latents and normalized codebook to d-major via PE transposes
  - sim tiles [128 latents x 8192 codes] via fp32 PE matmuls (exact fp32 class)
  - argmax per row via hardware InstMax (top-8) + InstMaxIndex
  - indices DMA'd out; host gathers codebook rows, computes straight-through
    output and the scalar loss from per-core partials (the "all-reduce").
"""
import os
import sys
import types

import numpy as np

_TRN_REPO = "/opt/trn_rl_repo"
if _TRN_REPO not in sys.path:
    sys.path.insert(0, _TRN_REPO)

import concourse.bass as bass
import concourse.mybir as mybir
from concourse import tile
from concourse.tile import ScopedClock

N_CORES = 8
N_FULL = 1024 * 32          # 32768 latent rows
N_SHARD = N_FULL // N_CORES  # 4096
D = 64
N_E = 8192
N_TILES = N_SHARD // 128     # 32 latent tiles per core
EPS = 1e-12
BETA = 0.25


def _install_profhook():
    """Re-register the NTFF profile hook trn_boot skips (antenv.axon_hooks
    is absent in this image) and stub the S3 artifact upload."""
    import antenv
    if "antenv.axon_hooks" not in sys.modules:
        mod = types.ModuleType("antenv.axon_hooks")
        _hook = [None]
        mod.set_axon_ntff_profile_hook = lambda h: _hook.__setitem__(0, h)
        mod.get_axon_ntff_profile_hook = lambda: _hook[0]
        sys.modules["antenv.axon_hooks"] = mod
        antenv.axon_hooks = mod
        try:
            from trn_agent_boot.trn_boot import _ntff_profile_via_ctypes
            mod.set_axon_ntff_profile_hook(
                _ntff_profile_via_ctypes("/opt/axon/libaxon_pjrt.so"))
        except Exception:
            pass
    import concourse.bass_utils as bu
    bu.upload_artifacts = lambda tmpdir: tmpdir


def _install_tilepatch():
    """This walrus build allows only ONE sync wait per TPB instruction; Tile's
    tail drain carries one wait per outstanding proc. Split them onto
    single-wait SP nops."""
    def _drain_and_barrier(self, tick_clock, wait_clock):
        nc = self.nc
        drain_inst = nc.sync.drain()
        wait_clock.add_sem_waits(
            drain_inst.ins, ScopedClock({None: tick_clock.global_clock}))
        si = drain_inst.ins.sync_info
        waits = list(si.on_wait or [])
        if len(waits) > 1:
            bb = nc.cur_bb.bb
            idx = bb.instructions.index(drain_inst.ins)
            nops = []
            for w in waits[:-1]:
                n = nc.sync.nop(nofuse=True)
                n.ins.sync_info = mybir.SyncInfo(on_wait=[w], on_update=[])
                nops.append(n.ins)
            si.on_wait = [waits[-1]]
            for n in nops:
                bb.instructions.remove(n)
            bb.instructions[idx:idx] = nops
        nc.all_engine_barrier()
        assert self.sems is not None
        popped = nc._tile_sem_poison_stack.pop()
        assert popped is self._sem_poison
        nc.clear_and_free_semaphores(list(self.sems.allocated().values()))
        nc.all_engine_barrier()

    tile.TileContext._drain_and_barrier = _drain_and_barrier


def _split_waits(nc, max_waits=1):
    """Hoist extra sync waits onto same-engine single-wait NOPs (same-engine
    FIFO makes this equivalent)."""
    n_split = 0
    for f in nc.m.functions:
        for bb in f.blocks:
            new_list = []
            for ins in bb.instructions:
                si = ins.sync_info
                waits = list(si.on_wait) if si and si.on_wait else []
                if len(waits) > max_waits:
                    eng = nc.engines[ins.engine]
                    extra, keep = waits[:-max_waits], waits[-max_waits:]
                    for w in extra:
                        nop = eng.nop(nofuse=True)
                        nopi = nop.ins
                        for bb2 in f.blocks:
                            if nopi in bb2.instructions:
                                bb2.instructions.remove(nopi)
                                break
                        nopi.sync_info = mybir.SyncInfo(on_wait=[w], on_update=[])
                        new_list.append(nopi)
                        n_split += 1
                    si.on_wait = keep
                new_list.append(ins)
            bb.instructions[:] = new_list
    return n_split


def _build_module():
    A = mybir.AluOpType
    f32 = mybir.dt.float32
    nc = bass.Bass()
    x_d = nc.dram_tensor("x", [N_SHARD, D], f32, kind="ExternalInput")
    emb_d = nc.dram_tensor("emb", [N_E, D], f32, kind="ExternalInput")
    id_d = nc.dram_tensor("ident", [128, 128], f32, kind="ExternalInput")
    idx_d = nc.dram_tensor("idx", [N_SHARD, 1], mybir.dt.int32,
                           kind="ExternalOutput")

    with tile.TileContext(nc) as tc:
        with tc.tile_pool(name="sb", bufs=1) as pool:
            ident = pool.tile([128, 128], f32)
            nc.sync.dma_start(ident[:], id_d[:])

            # ---- load emb as [128, 64*64] (tile t at cols t*64:(t+1)*64) ----
            emb_sb = pool.tile([128, 64 * D], f32)
            nc.sync.dma_start(
                emb_sb[:].rearrange("p (t d) -> p t d", t=64),
                emb_d[:].rearrange("(t p) d -> p t d", p=128))
            # ---- load x as [128, 32*64] ----
            x_sb = pool.tile([128, N_TILES * D], f32)
            nc.sync.dma_start(
                x_sb[:].rearrange("p (t d) -> p t d", t=N_TILES),
                x_d[:].rearrange("(t p) d -> p t d", p=128))

            # ---- codebook norms ----
            sq = pool.tile([128, 64 * D], f32)
            nc.scalar.activation(sq[:], emb_sb[:],
                                 mybir.ActivationFunctionType.Square)
            n2 = pool.tile([128, 64], f32)
            nc.vector.tensor_reduce(
                n2[:], sq[:].rearrange("p (t d) -> p t d", t=64),
                op=A.add, axis=mybir.AxisListType.X)
            nrm = pool.tile([128, 64], f32)
            nc.scalar.activation(nrm[:], n2[:],
                                 mybir.ActivationFunctionType.Sqrt)
            nc.vector.tensor_scalar_max(nrm[:], nrm[:], EPS)
            rn = pool.tile([128, 64], f32)
            nc.vector.reciprocal(rn[:], nrm[:])

            # ---- normalize + transpose codebook -> cbT [128, 8192]
            # (low/high partition halves hold the SAME data so two
            #  tile_position row-groups can stream concurrently) ----
            cbT = pool.tile([128, N_E], f32)
            latT = pool.tile([128, N_TILES * 128], f32)
            with tc.tile_pool(name="pst", bufs=4, space="PSUM") as pstpool:
                for t in range(64):
                    cn = pool.tile([128, D], f32, name=f"cn{t%4}", tag="cn", bufs=4)
                    nc.vector.tensor_scalar_mul(cn[:], emb_sb[:, t * D:(t + 1) * D],
                                                rn[:, t:t + 1])
                    pt = pstpool.tile([64, 128], f32, name=f"pt{t%4}", tag="pt")
                    nc.tensor.transpose(pt[:], cn[:], ident[:])
                    nc.scalar.copy(cbT[0:64, t * 128:(t + 1) * 128], pt[:])

                # ---- transpose latents -> latT [64, 32*128] ----
                for i in range(N_TILES):
                    pt2 = pstpool.tile([64, 128], f32, name=f"pt2{i%4}", tag="pt")
                    nc.tensor.transpose(pt2[:], x_sb[:, i * D:(i + 1) * D], ident[:])
                    nc.scalar.copy(latT[0:64, i * 128:(i + 1) * 128], pt2[:])
                # duplicate low halves into partitions 64-127
                nc.sync.dma_start(cbT[64:128, :], cbT[0:64, :])
                nc.sync.dma_start(latT[64:128, :], latT[0:64, :])

            # ---- main loop: sims + argmax ----
            with tc.tile_pool(name="ps", bufs=4, space="PSUM") as pspool:
              for i in range(N_TILES):
                sims = pool.tile([128, N_E], f32, name=f"sims{i % 2}",
                                 tag="sims", bufs=2)
                lhsT_lo = latT[0:64, i * 128:(i + 1) * 128]
                lhsT_hi = latT[64:128, i * 128:(i + 1) * 128]
                for r in range(8):
                    cA, cB = r, 8 + r
                    psA = pspool.tile([128, 512], f32, name=f"psA{r%4}",
                                      tag="psum_a", bufs=4)
                    psB = pspool.tile([128, 512], f32, name=f"psB{r%4}",
                                      tag="psum_b", bufs=4)
                    nc.tensor.matmul(psA[:], lhsT_lo,
                                     cbT[0:64, cA * 512:(cA + 1) * 512],
                                     start=True, stop=True,
                                     tile_position=(0, 0))
                    nc.tensor.matmul(psB[:], lhsT_hi,
                                     cbT[64:128, cB * 512:(cB + 1) * 512],
                                     start=True, stop=True,
                                     tile_position=(64, 0))
                    if r % 4 != 3:
                        nc.scalar.copy(sims[:, cA * 512:(cA + 1) * 512], psA[:])
                        nc.scalar.copy(sims[:, cB * 512:(cB + 1) * 512], psB[:])
                    else:
                        nc.vector.tensor_copy(sims[:, cA * 512:(cA + 1) * 512],
                                              psA[:])
                        nc.vector.tensor_copy(sims[:, cB * 512:(cB + 1) * 512],
                                              psB[:])
                mx8 = pool.tile([128, 8], f32, name=f"mx8_{i%4}", tag="mx8",
                                bufs=4)
                ix8 = pool.tile([128, 8], mybir.dt.uint32, name=f"ix8_{i%4}",
                                tag="ix8", bufs=4)
                nc.vector.max(mx8[:], sims[:])
                nc.vector.max_index(ix8[:], mx8[:], sims[:])
                nc.sync.dma_start(idx_d[i * 128:(i + 1) * 128, :],
                                  ix8[:, 0:1].bitcast(mybir.dt.int32))

    return nc


_CACHE = {}


def _get_kernel():
    if "nc" not in _CACHE:
        _install_profhook()
        _install_tilepatch()
        nc = _build_module()
        _split_waits(nc)
        _CACHE["nc"] = nc
    return _CACHE["nc"]


last_results = None


def kernel(x: np.ndarray, emb_weight: np.ndarray):
    global last_results
    from concourse.bass_utils import run_bass_kernel_spmd

    nc = _get_kernel()
    x = np.asarray(x, dtype=np.float32)
    emb = np.asarray(emb_weight, dtype=np.float32)
    flat = x.reshape(-1, D)
    ident = np.eye(128, dtype=np.float32)
    in_maps = [
        {"x": np.ascontiguousarray(flat[c * N_SHARD:(c + 1) * N_SHARD]),
         "emb": emb, "ident": ident}
        for c in range(N_CORES)
    ]
    trace = bool(os.environ.get("VQ_TRACE"))
    res = run_bass_kernel_spmd(nc, in_maps, core_ids=list(range(N_CORES)),
                               trace=trace)
    last_results = res
    idx = np.concatenate([res.results[c]["idx"][:, 0] for c in range(N_CORES)])
    idx = idx.astype(np.int32)

    codebook_vec = emb[idx].reshape(x.shape)
    x_q_st = x + (codebook_vec - x)
    diff = codebook_vec - x
    mse = np.float32(np.mean((diff * diff).astype(np.float32), dtype=np.float32))
    loss = np.float32(mse + np.float32(BETA) * mse)
    indices_out = idx.reshape(x.shape[:-1])
    return (x_q_st, loss, indices_out, codebook_vec)


# revision 7
# speedup vs baseline: 1.8826x; 1.1116x over previous
"""Cosine VQ (codebook) kernel for Trainium2, 8 NeuronCores, data-parallel over N.

Strategy per core (N_shard = 4096 latent rows, full codebook replicated):
  - normalize codebook rows on device (square -> windowed-reduce -> sqrt -> recip -> scale)
  - transpose latents and normalized codebook to d-major via PE transposes
  - sim tiles [128 latents x 8192 codes] via fp32 PE matmuls (exact fp32 class)
  - argmax per row via hardware InstMax (top-8) + InstMaxIndex
  - indices DMA'd out; host gathers codebook rows, computes straight-through
    output and the scalar loss from per-core partials (the "all-reduce").
"""
import os
import sys
import types

import numpy as np

_TRN_REPO = "/opt/trn_rl_repo"
if _TRN_REPO not in sys.path:
    sys.path.insert(0, _TRN_REPO)

import concourse.bass as bass
import concourse.mybir as mybir
from concourse import tile
from concourse.tile import ScopedClock

N_CORES = 8
N_FULL = 1024 * 32          # 32768 latent rows
N_SHARD = N_FULL // N_CORES  # 4096
D = 64
N_E = 8192
N_TILES = N_SHARD // 128     # 32 latent tiles per core
EPS = 1e-12
BETA = 0.25


def _install_profhook():
    """Re-register the NTFF profile hook trn_boot skips (antenv.axon_hooks
    is absent in this image) and stub the S3 artifact upload."""
    import antenv
    if "antenv.axon_hooks" not in sys.modules:
        mod = types.ModuleType("antenv.axon_hooks")
        _hook = [None]
        mod.set_axon_ntff_profile_hook = lambda h: _hook.__setitem__(0, h)
        mod.get_axon_ntff_profile_hook = lambda: _hook[0]
        sys.modules["antenv.axon_hooks"] = mod
        antenv.axon_hooks = mod
        try:
            from trn_agent_boot.trn_boot import _ntff_profile_via_ctypes
            mod.set_axon_ntff_profile_hook(
                _ntff_profile_via_ctypes("/opt/axon/libaxon_pjrt.so"))
        except Exception:
            pass
    import concourse.bass_utils as bu
    bu.upload_artifacts = lambda tmpdir: tmpdir


def _install_tilepatch():
    """This walrus build allows only ONE sync wait per TPB instruction; Tile's
    tail drain carries one wait per outstanding proc. Split them onto
    single-wait SP nops."""
    def _drain_and_barrier(self, tick_clock, wait_clock):
        nc = self.nc
        drain_inst = nc.sync.drain()
        wait_clock.add_sem_waits(
            drain_inst.ins, ScopedClock({None: tick_clock.global_clock}))
        si = drain_inst.ins.sync_info
        waits = list(si.on_wait or [])
        if len(waits) > 1:
            bb = nc.cur_bb.bb
            idx = bb.instructions.index(drain_inst.ins)
            nops = []
            for w in waits[:-1]:
                n = nc.sync.nop(nofuse=True)
                n.ins.sync_info = mybir.SyncInfo(on_wait=[w], on_update=[])
                nops.append(n.ins)
            si.on_wait = [waits[-1]]
            for n in nops:
                bb.instructions.remove(n)
            bb.instructions[idx:idx] = nops
        nc.all_engine_barrier()
        assert self.sems is not None
        popped = nc._tile_sem_poison_stack.pop()
        assert popped is self._sem_poison
        nc.clear_and_free_semaphores(list(self.sems.allocated().values()))
        nc.all_engine_barrier()

    tile.TileContext._drain_and_barrier = _drain_and_barrier


def _split_waits(nc, max_waits=1):
    """Hoist extra sync waits onto same-engine single-wait NOPs (same-engine
    FIFO makes this equivalent)."""
    n_split = 0
    for f in nc.m.functions:
        for bb in f.blocks:
            new_list = []
            for ins in bb.instructions:
                si = ins.sync_info
                waits = list(si.on_wait) if si and si.on_wait else []
                if len(waits) > max_waits:
                    eng = nc.engines[ins.engine]
                    extra, keep = waits[:-max_waits], waits[-max_waits:]
                    for w in extra:
                        nop = eng.nop(nofuse=True)
                        nopi = nop.ins
                        for bb2 in f.blocks:
                            if nopi in bb2.instructions:
                                bb2.instructions.remove(nopi)
                                break
                        nopi.sync_info = mybir.SyncInfo(on_wait=[w], on_update=[])
                        new_list.append(nopi)
                        n_split += 1
                    si.on_wait = keep
                new_list.append(ins)
            bb.instructions[:] = new_list
    return n_split


def _build_module():
    A = mybir.AluOpType
    f32 = mybir.dt.float32
    nc = bass.Bass()
    x_d = nc.dram_tensor("x", [N_SHARD, D], f32, kind="ExternalInput")
    emb_d = nc.dram_tensor("emb", [N_E, D], f32, kind="ExternalInput")
    id_d = nc.dram_tensor("ident", [128, 128], f32, kind="ExternalInput")
    idx_d = nc.dram_tensor("idx", [N_SHARD, 1], mybir.dt.int32,
                           kind="ExternalOutput")

    with tile.TileContext(nc) as tc:
        with tc.tile_pool(name="sb", bufs=1) as pool:
            ident = pool.tile([128, 128], f32)
            nc.sync.dma_start(ident[:], id_d[:])

            # ---- load emb as [128, 64*64] (tile t at cols t*64:(t+1)*64) ----
            emb_sb = pool.tile([128, 64 * D], f32)
            nc.sync.dma_start(
                emb_sb[:].rearrange("p (t d) -> p t d", t=64),
                emb_d[:].rearrange("(t p) d -> p t d", p=128))
            # ---- load x as [128, 32*64] ----
            x_sb = pool.tile([128, N_TILES * D], f32)
            nc.sync.dma_start(
                x_sb[:].rearrange("p (t d) -> p t d", t=N_TILES),
                x_d[:].rearrange("(t p) d -> p t d", p=128))

            # ---- codebook norms ----
            sq = pool.tile([128, 64 * D], f32)
            nc.scalar.activation(sq[:], emb_sb[:],
                                 mybir.ActivationFunctionType.Square)
            n2 = pool.tile([128, 64], f32)
            nc.vector.tensor_reduce(
                n2[:], sq[:].rearrange("p (t d) -> p t d", t=64),
                op=A.add, axis=mybir.AxisListType.X)
            nrm = pool.tile([128, 64], f32)
            nc.scalar.activation(nrm[:], n2[:],
                                 mybir.ActivationFunctionType.Sqrt)
            nc.vector.tensor_scalar_max(nrm[:], nrm[:], EPS)
            rn = pool.tile([128, 64], f32)
            nc.vector.reciprocal(rn[:], nrm[:])

            # ---- normalize + transpose codebook -> cbT [128, 8192]
            # (low/high partition halves hold the SAME data so two
            #  tile_position row-groups can stream concurrently) ----
            cbT = pool.tile([128, N_E], f32)
            latT = pool.tile([128, N_TILES * 128], f32)
            with tc.tile_pool(name="pst", bufs=4, space="PSUM") as pstpool:
                for t in range(64):
                    cn = pool.tile([128, D], f32, name=f"cn{t%4}", tag="cn", bufs=4)
                    nc.vector.tensor_scalar_mul(cn[:], emb_sb[:, t * D:(t + 1) * D],
                                                rn[:, t:t + 1])
                    pt = pstpool.tile([64, 128], f32, name=f"pt{t%4}", tag="pt")
                    nc.tensor.transpose(pt[:], cn[:], ident[:])
                    nc.scalar.copy(cbT[0:64, t * 128:(t + 1) * 128], pt[:])

                # ---- transpose latents -> latT [64, 32*128] ----
                for i in range(N_TILES):
                    pt2 = pstpool.tile([64, 128], f32, name=f"pt2{i%4}", tag="pt")
                    nc.tensor.transpose(pt2[:], x_sb[:, i * D:(i + 1) * D], ident[:])
                    nc.scalar.copy(latT[0:64, i * 128:(i + 1) * 128], pt2[:])
                # duplicate low halves into partitions 64-127
                nc.sync.dma_start(cbT[64:128, :], cbT[0:64, :])
                nc.sync.dma_start(latT[64:128, :], latT[0:64, :])

            # ---- main loop: sims + argmax ----
            with tc.tile_pool(name="ps", bufs=4, space="PSUM") as pspool:
              for i in range(N_TILES):
                sims = pool.tile([128, N_E], f32, name=f"sims{i % 2}",
                                 tag="sims", bufs=2)
                lhsT_lo = latT[0:64, i * 128:(i + 1) * 128]
                lhsT_hi = latT[64:128, i * 128:(i + 1) * 128]
                for r in range(8):
                    cA, cB = r, 8 + r
                    psA = pspool.tile([128, 512], f32, name=f"psA{r%4}",
                                      tag="psum_a", bufs=4)
                    psB = pspool.tile([128, 512], f32, name=f"psB{r%4}",
                                      tag="psum_b", bufs=4)
                    nc.tensor.matmul(psA[:], lhsT_lo,
                                     cbT[0:64, cA * 512:(cA + 1) * 512],
                                     start=True, stop=True,
                                     tile_position=(0, 0))
                    nc.tensor.matmul(psB[:], lhsT_hi,
                                     cbT[64:128, cB * 512:(cB + 1) * 512],
                                     start=True, stop=True,
                                     tile_position=(64, 0))
                    nc.scalar.copy(sims[:, cA * 512:(cA + 1) * 512], psA[:])
                    nc.scalar.copy(sims[:, cB * 512:(cB + 1) * 512], psB[:])
                mx8 = pool.tile([128, 8], f32, name=f"mx8_{i%4}", tag="mx8",
                                bufs=4)
                ix8 = pool.tile([128, 8], mybir.dt.uint32, name=f"ix8_{i%4}",
                                tag="ix8", bufs=4)
                nc.vector.max(mx8[:], sims[:])
                nc.vector.max_index(ix8[:], mx8[:], sims[:])
                nc.sync.dma_start(idx_d[i * 128:(i + 1) * 128, :],
                                  ix8[:, 0:1].bitcast(mybir.dt.int32))

    return nc


_CACHE = {}


def _get_kernel():
    if "nc" not in _CACHE:
        _install_profhook()
        _install_tilepatch()
        nc = _build_module()
        _split_waits(nc)
        _CACHE["nc"] = nc
    return _CACHE["nc"]


last_results = None


def kernel(x: np.ndarray, emb_weight: np.ndarray):
    global last_results
    from concourse.bass_utils import run_bass_kernel_spmd

    nc = _get_kernel()
    x = np.asarray(x, dtype=np.float32)
    emb = np.asarray(emb_weight, dtype=np.float32)
    flat = x.reshape(-1, D)
    ident = np.eye(128, dtype=np.float32)
    in_maps = [
        {"x": np.ascontiguousarray(flat[c * N_SHARD:(c + 1) * N_SHARD]),
         "emb": emb, "ident": ident}
        for c in range(N_CORES)
    ]
    trace = bool(os.environ.get("VQ_TRACE"))
    res = run_bass_kernel_spmd(nc, in_maps, core_ids=list(range(N_CORES)),
                               trace=trace)
    last_results = res
    idx = np.concatenate([res.results[c]["idx"][:, 0] for c in range(N_CORES)])
    idx = idx.astype(np.int32)

    codebook_vec = emb[idx].reshape(x.shape)
    x_q_st = x + (codebook_vec - x)
    diff = codebook_vec - x
    mse = np.float32(np.mean((diff * diff).astype(np.float32), dtype=np.float32))
    loss = np.float32(mse + np.float32(BETA) * mse)
    indices_out = idx.reshape(x.shape[:-1])
    return (x_q_st, loss, indices_out, codebook_vec)


# revision 8
# speedup vs baseline: 1.8952x; 1.0067x over previous
"""Cosine VQ (codebook) kernel for Trainium2, 8 NeuronCores, data-parallel over N.

Strategy per core (N_shard = 4096 latent rows, full codebook replicated):
  - normalize codebook rows on device (square -> windowed-reduce -> sqrt -> recip -> scale)
  - transpose latents and normalized codebook to d-major via PE transposes
  - sim tiles [128 latents x 8192 codes] via fp32 PE matmuls (exact fp32 class)
  - argmax per row via hardware InstMax (top-8) + InstMaxIndex
  - indices DMA'd out; host gathers codebook rows, computes straight-through
    output and the scalar loss from per-core partials (the "all-reduce").
"""
import os
import sys
import types

import numpy as np

_TRN_REPO = "/opt/trn_rl_repo"
if _TRN_REPO not in sys.path:
    sys.path.insert(0, _TRN_REPO)

import concourse.bass as bass
import concourse.mybir as mybir
from concourse import tile
from concourse.tile import ScopedClock

N_CORES = 8
N_FULL = 1024 * 32          # 32768 latent rows
N_SHARD = N_FULL // N_CORES  # 4096
D = 64
N_E = 8192
N_TILES = N_SHARD // 128     # 32 latent tiles per core
EPS = 1e-12
BETA = 0.25


def _install_profhook():
    """Re-register the NTFF profile hook trn_boot skips (antenv.axon_hooks
    is absent in this image) and stub the S3 artifact upload."""
    import antenv
    if "antenv.axon_hooks" not in sys.modules:
        mod = types.ModuleType("antenv.axon_hooks")
        _hook = [None]
        mod.set_axon_ntff_profile_hook = lambda h: _hook.__setitem__(0, h)
        mod.get_axon_ntff_profile_hook = lambda: _hook[0]
        sys.modules["antenv.axon_hooks"] = mod
        antenv.axon_hooks = mod
        try:
            from trn_agent_boot.trn_boot import _ntff_profile_via_ctypes
            mod.set_axon_ntff_profile_hook(
                _ntff_profile_via_ctypes("/opt/axon/libaxon_pjrt.so"))
        except Exception:
            pass
    import concourse.bass_utils as bu
    bu.upload_artifacts = lambda tmpdir: tmpdir


def _install_tilepatch():
    """This walrus build allows only ONE sync wait per TPB instruction; Tile's
    tail drain carries one wait per outstanding proc. Split them onto
    single-wait SP nops."""
    def _drain_and_barrier(self, tick_clock, wait_clock):
        nc = self.nc
        drain_inst = nc.sync.drain()
        wait_clock.add_sem_waits(
            drain_inst.ins, ScopedClock({None: tick_clock.global_clock}))
        si = drain_inst.ins.sync_info
        waits = list(si.on_wait or [])
        if len(waits) > 1:
            bb = nc.cur_bb.bb
            idx = bb.instructions.index(drain_inst.ins)
            nops = []
            for w in waits[:-1]:
                n = nc.sync.nop(nofuse=True)
                n.ins.sync_info = mybir.SyncInfo(on_wait=[w], on_update=[])
                nops.append(n.ins)
            si.on_wait = [waits[-1]]
            for n in nops:
                bb.instructions.remove(n)
            bb.instructions[idx:idx] = nops
        nc.all_engine_barrier()
        assert self.sems is not None
        popped = nc._tile_sem_poison_stack.pop()
        assert popped is self._sem_poison
        nc.clear_and_free_semaphores(list(self.sems.allocated().values()))
        nc.all_engine_barrier()

    tile.TileContext._drain_and_barrier = _drain_and_barrier


def _split_waits(nc, max_waits=1):
    """Hoist extra sync waits onto same-engine single-wait NOPs (same-engine
    FIFO makes this equivalent)."""
    n_split = 0
    for f in nc.m.functions:
        for bb in f.blocks:
            new_list = []
            for ins in bb.instructions:
                si = ins.sync_info
                waits = list(si.on_wait) if si and si.on_wait else []
                if len(waits) > max_waits:
                    eng = nc.engines[ins.engine]
                    extra, keep = waits[:-max_waits], waits[-max_waits:]
                    for w in extra:
                        nop = eng.nop(nofuse=True)
                        nopi = nop.ins
                        for bb2 in f.blocks:
                            if nopi in bb2.instructions:
                                bb2.instructions.remove(nopi)
                                break
                        nopi.sync_info = mybir.SyncInfo(on_wait=[w], on_update=[])
                        new_list.append(nopi)
                        n_split += 1
                    si.on_wait = keep
                new_list.append(ins)
            bb.instructions[:] = new_list
    return n_split


def _build_module():
    A = mybir.AluOpType
    f32 = mybir.dt.float32
    nc = bass.Bass()
    x_d = nc.dram_tensor("x", [N_SHARD, D], f32, kind="ExternalInput")
    emb_d = nc.dram_tensor("emb", [N_E, D], f32, kind="ExternalInput")
    id_d = nc.dram_tensor("ident", [128, 128], f32, kind="ExternalInput")
    idx_d = nc.dram_tensor("idx", [N_SHARD, 1], mybir.dt.int32,
                           kind="ExternalOutput")

    with tile.TileContext(nc) as tc:
        with tc.tile_pool(name="sb", bufs=1) as pool:
            ident = pool.tile([128, 128], f32)
            nc.sync.dma_start(ident[:], id_d[:])

            # ---- load emb as [128, 64*64] (tile t at cols t*64:(t+1)*64) ----
            emb_sb = pool.tile([128, 64 * D], f32)
            nc.sync.dma_start(
                emb_sb[:].rearrange("p (t d) -> p t d", t=64),
                emb_d[:].rearrange("(t p) d -> p t d", p=128))
            # ---- load x as [128, 32*64] ----
            x_sb = pool.tile([128, N_TILES * D], f32)
            nc.sync.dma_start(
                x_sb[:].rearrange("p (t d) -> p t d", t=N_TILES),
                x_d[:].rearrange("(t p) d -> p t d", p=128))

            # ---- codebook norms ----
            sq = pool.tile([128, 64 * D], f32)
            nc.scalar.activation(sq[:], emb_sb[:],
                                 mybir.ActivationFunctionType.Square)
            n2 = pool.tile([128, 64], f32)
            nc.vector.tensor_reduce(
                n2[:], sq[:].rearrange("p (t d) -> p t d", t=64),
                op=A.add, axis=mybir.AxisListType.X)
            nrm = pool.tile([128, 64], f32)
            nc.scalar.activation(nrm[:], n2[:],
                                 mybir.ActivationFunctionType.Sqrt)
            nc.vector.tensor_scalar_max(nrm[:], nrm[:], EPS)
            rn = pool.tile([128, 64], f32)
            nc.vector.reciprocal(rn[:], nrm[:])

            # ---- normalize + transpose codebook -> cbT [128, 8192]
            # (low/high partition halves hold the SAME data so two
            #  tile_position row-groups can stream concurrently) ----
            cbT = pool.tile([128, N_E], f32)
            latT = pool.tile([128, N_TILES * 128], f32)
            with tc.tile_pool(name="pst", bufs=4, space="PSUM") as pstpool:
                for t in range(64):
                    cn = pool.tile([128, D], f32, name=f"cn{t%4}", tag="cn", bufs=4)
                    nc.vector.tensor_scalar_mul(cn[:], emb_sb[:, t * D:(t + 1) * D],
                                                rn[:, t:t + 1])
                    pt = pstpool.tile([64, 128], f32, name=f"pt{t%4}", tag="pt")
                    nc.tensor.transpose(pt[:], cn[:], ident[:])
                    nc.scalar.copy(cbT[0:64, t * 128:(t + 1) * 128], pt[:])

                # ---- transpose latents -> latT [64, 32*128] ----
                for i in range(N_TILES):
                    pt2 = pstpool.tile([64, 128], f32, name=f"pt2{i%4}", tag="pt")
                    nc.tensor.transpose(pt2[:], x_sb[:, i * D:(i + 1) * D], ident[:])
                    nc.scalar.copy(latT[0:64, i * 128:(i + 1) * 128], pt2[:])
                # duplicate low halves into partitions 64-127
                nc.sync.dma_start(cbT[64:128, :], cbT[0:64, :])
                nc.sync.dma_start(latT[64:128, :], latT[0:64, :])

            # ---- main loop: sims + argmax ----
            with tc.tile_pool(name="ps", bufs=4, space="PSUM") as pspool:
              for i in range(N_TILES):
                sims = pool.tile([128, N_E], f32, name=f"sims{i % 3}",
                                 tag="sims", bufs=3)
                lhsT_lo = latT[0:64, i * 128:(i + 1) * 128]
                lhsT_hi = latT[64:128, i * 128:(i + 1) * 128]
                for r in range(8):
                    cA, cB = r, 8 + r
                    psA = pspool.tile([128, 512], f32, name=f"psA{r%4}",
                                      tag="psum_a", bufs=4)
                    psB = pspool.tile([128, 512], f32, name=f"psB{r%4}",
                                      tag="psum_b", bufs=4)
                    nc.tensor.matmul(psA[:], lhsT_lo,
                                     cbT[0:64, cA * 512:(cA + 1) * 512],
                                     start=True, stop=True,
                                     tile_position=(0, 0))
                    nc.tensor.matmul(psB[:], lhsT_hi,
                                     cbT[64:128, cB * 512:(cB + 1) * 512],
                                     start=True, stop=True,
                                     tile_position=(64, 0))
                    nc.scalar.copy(sims[:, cA * 512:(cA + 1) * 512], psA[:])
                    nc.scalar.copy(sims[:, cB * 512:(cB + 1) * 512], psB[:])
                mx8 = pool.tile([128, 8], f32, name=f"mx8_{i%4}", tag="mx8",
                                bufs=4)
                ix8 = pool.tile([128, 8], mybir.dt.uint32, name=f"ix8_{i%4}",
                                tag="ix8", bufs=4)
                nc.vector.max(mx8[:], sims[:])
                nc.vector.max_index(ix8[:], mx8[:], sims[:])
                nc.sync.dma_start(idx_d[i * 128:(i + 1) * 128, :],
                                  ix8[:, 0:1].bitcast(mybir.dt.int32))

    return nc


_CACHE = {}


def _get_kernel():
    if "nc" not in _CACHE:
        _install_profhook()
        _install_tilepatch()
        nc = _build_module()
        _split_waits(nc)
        _CACHE["nc"] = nc
    return _CACHE["nc"]


last_results = None


def kernel(x: np.ndarray, emb_weight: np.ndarray):
    global last_results
    from concourse.bass_utils import run_bass_kernel_spmd

    nc = _get_kernel()
    x = np.asarray(x, dtype=np.float32)
    emb = np.asarray(emb_weight, dtype=np.float32)
    flat = x.reshape(-1, D)
    ident = np.eye(128, dtype=np.float32)
    in_maps = [
        {"x": np.ascontiguousarray(flat[c * N_SHARD:(c + 1) * N_SHARD]),
         "emb": emb, "ident": ident}
        for c in range(N_CORES)
    ]
    trace = bool(os.environ.get("VQ_TRACE"))
    res = run_bass_kernel_spmd(nc, in_maps, core_ids=list(range(N_CORES)),
                               trace=trace)
    last_results = res
    idx = np.concatenate([res.results[c]["idx"][:, 0] for c in range(N_CORES)])
    idx = idx.astype(np.int32)

    codebook_vec = emb[idx].reshape(x.shape)
    x_q_st = x + (codebook_vec - x)
    diff = codebook_vec - x
    mse = np.float32(np.mean((diff * diff).astype(np.float32), dtype=np.float32))
    loss = np.float32(mse + np.float32(BETA) * mse)
    indices_out = idx.reshape(x.shape[:-1])
    return (x_q_st, loss, indices_out, codebook_vec)
